# revision 3
# baseline (speedup 1.0000x reference)
"""Trainium2 Bass kernel for nn_MixedHeadsV2 (mixed-head causal attention).

Full inputs in, full output out. Sharding: 8 cores = 4 batches x 2 head-groups.
Each core handles one batch and 4 of the 8 base heads: even cores heads
{0,1,4,5}, odd cores {2,3,6,7}. Heads 0-3 ("heavy") have effective head size
128; heads 4-7 ("light") have effective head size 64 (their mixed weight rows
64:128 are exactly zero), so the two light heads are packed into one 128-wide
tensor for projections.

Per-core pipeline (all on one NeuronCore, Tile-scheduled):
  1. Build effective-weight mixing patterns effA/effB from `weights` via two
     tiny K=4 matmuls against memset rank-1 masks.
  2. W = base * eff (DVE), PE-transpose to W^T (bf16).
  3. PE-transpose x to x^T (bf16).
  4. Projections q^T,k^T (d-major) and v (t-major, with a ones column fused
     for the softmax denominator).
  5. Causal attention: scoresT tiles [s128, t512] = k^T.T @ q^T, exp on ACT
     (scale folded in, no max-subtraction: |scaled scores| < 3), causal mask
     via precomputed affine_select masks, AV with fused row-sum, normalize.
"""
import sys

for p in ("/opt/trn_rl_repo",):
    if p not in sys.path:
        sys.path.append(p)

import numpy as np

import concourse.bass as bass
import concourse.tile as tile
from concourse import bacc, mybir
from concourse.bass_utils import run_bass_kernel_spmd

FP32 = mybir.dt.float32
BF16 = mybir.dt.bfloat16
AF = mybir.ActivationFunctionType
ALU = mybir.AluOpType

T = 2048
C = 512
HS = 128          # heavy head size (= padded head size)
LS = 64           # light head size
NT128 = T // 128  # 16
NT512 = T // 512  # 4
NCC = C // 128    # 4
SCALE = float(1.0 / np.sqrt(128.0))
SGRP = 3          # score tiles (512 wide) per exp group; 3 banks * 2 bufs + 2 = 8

# Units per core: (name, qk tensor idx, d_lo, d_hi, v tensor idx, v_lo, v_hi, out_col)
UNITS = [
    ("h0", 0, 0, 128, 0, 0, 129, 0),
    ("h1", 1, 0, 128, 1, 0, 129, 128),
    ("l0", 2, 0, 64, 2, 0, 65, 256),
    ("l1", 2, 64, 128, 2, 65, 130, 384),
]

_CACHE = {}


def _build():
    nc = bacc.Bacc("TRN2", target_bir_lowering=False, debug=False, num_devices=8)
    x_d = nc.dram_tensor("x", [T, C], FP32, kind="ExternalInput")
    w_d = nc.dram_tensor("w", [4, 1], FP32, kind="ExternalInput")
    bq_d = nc.dram_tensor("bq", [4, HS, C], FP32, kind="ExternalInput")
    bk_d = nc.dram_tensor("bk", [4, HS, C], FP32, kind="ExternalInput")
    bv_d = nc.dram_tensor("bv", [4, HS, C], FP32, kind="ExternalInput")
    out_d = nc.dram_tensor("out", [T, 4 * HS], FP32, kind="ExternalOutput")

    with tile.TileContext(nc) as tc:
        _emit(nc, tc, x_d, w_d, bq_d, bk_d, bv_d, out_d)
    nc.compile()
    return nc


def _emit(nc, tc, x_d, w_d, bq_d, bk_d, bv_d, out_d):
    from contextlib import ExitStack

    ctx = ExitStack()
    with ctx:
        # ---- persistent SBUF pools ----
        const_p = ctx.enter_context(tc.tile_pool(name="const", bufs=1))
        wts_p = ctx.enter_context(tc.tile_pool(name="wts", bufs=1))
        stage_p = ctx.enter_context(tc.tile_pool(name="stage", bufs=3))
        xt_p = ctx.enter_context(tc.tile_pool(name="xt", bufs=1))
        qk_p = ctx.enter_context(tc.tile_pool(name="qk", bufs=1))
        v_p = ctx.enter_context(tc.tile_pool(name="v", bufs=1))
        pt_p = ctx.enter_context(tc.tile_pool(name="pt", bufs=2))
        o_p = ctx.enter_context(tc.tile_pool(name="o", bufs=3))
        r_p = ctx.enter_context(tc.tile_pool(name="r", bufs=3))
        # ---- PSUM pools: 3-bank score groups x2 + 1-bank small x2 = 8 banks
        sps = ctx.enter_context(tc.tile_pool(name="sps", bufs=2, space="PSUM"))
        ps = ctx.enter_context(tc.tile_pool(name="ps", bufs=2, space="PSUM"))

        # ================= constants =================
        ones_f = const_p.tile([128, C], FP32, tag="ones_f")
        nc.vector.memset(ones_f[:], 1.0)
        ident_f = const_p.tile([128, 128], FP32, tag="ident_f")
        nc.gpsimd.affine_select(
            ident_f[:], ones_f[:, 0:128], pattern=[[1, 128]],
            compare_op=ALU.is_equal, fill=0.0, base=0, channel_multiplier=-1)
        ident_b = const_p.tile([128, 128], BF16, tag="ident_b")
        nc.vector.tensor_copy(ident_b[:], ident_f[:])
        ones_b = const_p.tile([128, C], BF16, tag="ones_b")
        nc.vector.tensor_copy(ones_b[:], ones_f[:])
        # causal masks for diagonal tiles: mask_r[s, t] = (t - s - 128*r >= 0)
        masks = []
        for r in range(4):
            m = const_p.tile([128, 512], BF16, tag=f"mask{r}")
            nc.gpsimd.affine_select(
                m[:], ones_b[:, 0:512], pattern=[[1, 512]],
                compare_op=ALU.is_ge, fill=0.0, base=-128 * r,
                channel_multiplier=-1)
            masks.append(m)

        # ================= eff patterns =================
        # effA[d, e] = sum_i w_i * (d < hs_i) * (e < emb_i)            (heads 0-3)
        # effB_ext[d, e] = same for i in {1,3} with (d%64 < hs_i)      (packed light)
        # Built as rank-1 K=1 accumulating matmuls; every piece lives on
        # partition 0 (compute engines can't address partition bases 1/3).
        HSL = (64, 32, 128, 64)   # hs per config
        EMB = (256, 256, 512, 512)
        wsc = []
        for i in range(4):
            wi = const_p.tile([1, 1], FP32, name=f"wsc{i}", tag=f"wsc{i}")
            nc.sync.dma_start(wi[:], w_d.ap()[i:i + 1, :])
            wsc.append(wi)
        effA = const_p.tile([128, C], FP32, tag="effA")
        effB = const_p.tile([128, C], FP32, tag="effB")
        for eff, cfgs, ext in ((effA, (0, 1, 2, 3), False), (effB, (1, 3), True)):
            p = ps.tile([128, 512], FP32, tag="ps")
            for n, i in enumerate(cfgs):
                u = stage_p.tile([1, 128], FP32, name=f"u{i}{ext}", tag="u_row")
                nc.vector.memset(u[:], 0.0)
                if ext:  # packed light: both 64-halves get the (d%64 < hs) pattern
                    nc.vector.memset(u[0:1, 0:min(HSL[i], 64)], 1.0)
                    nc.vector.memset(u[0:1, 64:64 + min(HSL[i], 64)], 1.0)
                else:
                    nc.vector.memset(u[0:1, 0:HSL[i]], 1.0)
                uw = stage_p.tile([1, 128], FP32, name=f"uw{i}{ext}", tag="uw_row")
                nc.vector.tensor_scalar_mul(uw[:], u[:], wsc[i][:])
                vrow = stage_p.tile([1, C], FP32, name=f"v{i}{ext}", tag="v_row")
                nc.vector.memset(vrow[:], 0.0)
                nc.vector.memset(vrow[0:1, 0:EMB[i]], 1.0)
                nc.tensor.matmul(p[:], uw[:], vrow[:],
                                 start=(n == 0), stop=(n == len(cfgs) - 1))
            nc.vector.tensor_copy(eff[:], p[:])

        # ================= effective weights, transposed =================
        # wt[j][cc]: [128c, 128d] bf16, j in 0..8 (q h0,h1,l | k ... | v ...)
        wt = [[wts_p.tile([128, 128], BF16, name=f"wt{j}_{cc}", tag=f"wt{j}_{cc}") for cc in range(NCC)]
              for j in range(9)]
        for pi, bd in enumerate((bq_d, bk_d, bv_d)):
            for hj in range(3):
                j = pi * 3 + hj
                w_bf = stage_p.tile([128, C], BF16, tag="w_bf")
                base = stage_p.tile([128, C], FP32, tag="base")
                if hj < 2:
                    nc.sync.dma_start(base[:], bd.ap()[hj])
                    nc.vector.tensor_mul(w_bf[:], base[:], effA[:])
                else:
                    nc.sync.dma_start(base[0:64, :], bd.ap()[2][0:64, :])
                    nc.sync.dma_start(base[64:128, :], bd.ap()[3][0:64, :])
                    nc.vector.tensor_mul(w_bf[:], base[:], effB[:])
                for cc in range(NCC):
                    pt = ps.tile([128, 512], BF16, tag="ps")
                    nc.tensor.transpose(
                        pt[:, 0:128], w_bf[:, cc * 128:(cc + 1) * 128], ident_b[:])
                    nc.vector.tensor_copy(wt[j][cc][:], pt[:, 0:128])

        # ================= x^T =================
        xt = [xt_p.tile([128, T], BF16, name=f"xt{cc}", tag=f"xt{cc}") for cc in range(NCC)]
        for tt in range(NT128):
            xs = stage_p.tile([128, C], FP32, tag="xs")
            nc.sync.dma_start(xs[:], x_d.ap()[tt * 128:(tt + 1) * 128, :])
            for cc in range(NCC):
                pt = ps.tile([128, 512], FP32, tag="ps")
                nc.tensor.transpose(
                    pt[:, 0:128], xs[:, cc * 128:(cc + 1) * 128], ident_f[:])
                nc.vector.tensor_copy(
                    xt[cc][:, tt * 128:(tt + 1) * 128], pt[:, 0:128])

        # ================= projections =================
        # q^T, k^T: [128d, T] bf16 per tensor (heavy0, heavy1, packed light)
        qt = [qk_p.tile([128, T], BF16, name=f"qt{h}", tag=f"qt{h}") for h in range(3)]
        kt = [qk_p.tile([128, T], BF16, name=f"kt{h}", tag=f"kt{h}") for h in range(3)]
        for dst, j0 in ((qt, 0), (kt, 3)):
            for hj in range(3):
                for tj in range(NT512):
                    p = ps.tile([128, 512], FP32, tag="ps")
                    for cc in range(NCC):
                        nc.tensor.matmul(
                            p[:], wt[j0 + hj][cc][:],
                            xt[cc][:, tj * 512:(tj + 1) * 512],
                            start=(cc == 0), stop=(cc == NCC - 1))
                    nc.vector.tensor_copy(
                        dst[hj][:, tj * 512:(tj + 1) * 512], p[:])
        # v tiles: heavy [128, 129] (ones col at 128); light packed [128, 130]
        # (head l0 cols 0:64 + ones at 64, head l1 cols 65:129 + ones at 129)
        vtiles = [[v_p.tile([128, 132], BF16, name=f"v{h}_{i}", tag=f"v{h}_{i}") for i in range(NT128)]
                  for h in range(3)]
        for hj in range(3):
            for i in range(NT128):
                p = ps.tile([128, 512], FP32, tag="ps")
                for cc in range(NCC):
                    nc.tensor.matmul(
                        p[:, 0:128], xt[cc][:, i * 128:(i + 1) * 128],
                        wt[6 + hj][cc][:],
                        start=(cc == 0), stop=(cc == NCC - 1))
                vt_i = vtiles[hj][i]
                if hj < 2:
                    nc.vector.tensor_copy(vt_i[:, 0:128], p[:, 0:128])
                    nc.vector.memset(vt_i[:, 128:129], 1.0)
                else:
                    nc.vector.tensor_copy(vt_i[:, 0:64], p[:, 0:64])
                    nc.vector.tensor_copy(vt_i[:, 65:129], p[:, 64:128])
                    nc.vector.memset(vt_i[:, 64:65], 1.0)
                    nc.vector.memset(vt_i[:, 129:130], 1.0)

        # ================= attention =================
        for (uname, qkj, d_lo, d_hi, vj, v_lo, v_hi, ocol) in UNITS:
            for tj in range(NT512):
                S = 4 * tj + 4  # s-chunks (128 wide) needed for this t-chunk
                ptile = pt_p.tile([128, 16 * 512], BF16, tag="pt")
                g = 0
                while g < S:
                    gw = min(SGRP, S - g)
                    sp = sps.tile([128, SGRP * 512], FP32, tag="sps")
                    for k in range(gw):
                        i = g + k
                        nc.tensor.matmul(
                            sp[:, k * 512:(k + 1) * 512],
                            kt[qkj][d_lo:d_hi, i * 128:(i + 1) * 128],
                            qt[qkj][d_lo:d_hi, tj * 512:(tj + 1) * 512],
                            start=True, stop=True)
                    nc.scalar.activation(
                        ptile[:, g * 512:(g + gw) * 512],
                        sp[:, 0:gw * 512], AF.Exp, scale=SCALE)
                    g += gw
                # causal masks on the 4 diagonal tiles
                for r in range(4):
                    i = 4 * tj + r
                    nc.vector.tensor_mul(
                        ptile[:, i * 512:(i + 1) * 512],
                        ptile[:, i * 512:(i + 1) * 512], masks[r][:])
                # AV + fused row-sum, normalize, store
                for m in range(4):
                    ti = 4 * tj + m  # global t128 index
                    w = v_hi - v_lo
                    op = ps.tile([128, 512], FP32, tag="ps")
                    for i in range(ti + 1):
                        nc.tensor.matmul(
                            op[:, 0:w],
                            ptile[:, i * 512 + m * 128: i * 512 + (m + 1) * 128],
                            vtiles[vj][i][:, v_lo:v_hi],
                            start=(i == 0), stop=(i == ti))
                    rec = r_p.tile([128, 1], FP32, tag="rec")
                    nc.vector.reciprocal(rec[:], op[:, w - 1:w])
                    ob = o_p.tile([128, 128], FP32, tag="ob")
                    nc.vector.tensor_scalar_mul(ob[:, 0:w - 1], op[:, 0:w - 1], rec[:])
                    nc.sync.dma_start(
                        out_d.ap()[ti * 128:(ti + 1) * 128, ocol:ocol + (w - 1)],
                        ob[:, 0:w - 1])


def _shard_inputs(x, weights, base_K, base_Q, base_V):
    in_maps = []
    for c in range(8):
        b = c // 2
        hsel = [0, 1, 4, 5] if c % 2 == 0 else [2, 3, 6, 7]
        in_maps.append({
            "x": np.ascontiguousarray(x[b]),
            "w": np.ascontiguousarray(weights.reshape(4, 1)),
            "bq": np.ascontiguousarray(base_Q[hsel]),
            "bk": np.ascontiguousarray(base_K[hsel]),
            "bv": np.ascontiguousarray(base_V[hsel]),
        })
    return in_maps


def _gather(results):
    out = np.zeros((4, T, 8 * HS), np.float32)
    for c in range(8):
        o = results[c]["out"]
        hsel = [0, 1, 4, 5] if c % 2 == 0 else [2, 3, 6, 7]
        for j, h in enumerate(hsel):
            out[c // 2][:, h * HS:(h + 1) * HS] = o[:, j * HS:(j + 1) * HS]
    return out


def get_nc():
    if "nc" not in _CACHE:
        _CACHE["nc"] = _build()
    return _CACHE["nc"]


def kernel(x, weights, base_K, base_Q, base_V):
    x = np.asarray(x, np.float32)
    weights = np.asarray(weights, np.float32)
    base_K = np.asarray(base_K, np.float32)
    base_Q = np.asarray(base_Q, np.float32)
    base_V = np.asarray(base_V, np.float32)
    nc = get_nc()
    in_maps = _shard_inputs(x, weights, base_K, base_Q, base_V)
    res = run_bass_kernel_spmd(nc, in_maps, core_ids=list(range(8)))
    return _gather(res.results)


# revision 4
# speedup vs baseline: 1.0121x; 1.0121x over previous
"""Trainium2 Bass kernel for nn_MixedHeadsV2 (mixed-head causal attention).

Full inputs in, full output out. Sharding: 8 cores = 4 batches x 2 head-groups.
Each core handles one batch and 4 of the 8 base heads: even cores heads
{0,1,4,5}, odd cores {2,3,6,7}. Heads 0-3 ("heavy") have effective head size
128; heads 4-7 ("light") have effective head size 64 (their mixed weight rows
64:128 are exactly zero), so the two light heads are packed into one 128-wide
tensor for projections and run concurrently on disjoint PE row groups in
attention.

Per-core pipeline (all on one NeuronCore, Tile-scheduled):
  1. Build effective-weight mixing patterns effA/effB from `weights` via tiny
     rank-1 K=1 matmuls against memset masks (bf16).
  2. x -> bf16 -> DRAM scratch -> DMA-transpose to x^T (xbar, frees PE).
  3. W = base * eff (DVE), PE-transpose to W^T (bf16).
  4. Projections q^T,k^T (d-major; PSUM->SBUF copies on ScalarE, idle in this
     phase) and v (t-major, ones column fused for the softmax denominator).
  5. Causal attention, scoresT layout [s128, t512]: scores = k^T.T @ q^T into
     3-bank PSUM groups, exp on ACT (scale folded; no max-subtraction:
     |scaled scores| < 3), causal masking of diagonal tiles via precomputed
     affine_select masks, AV with fused row-sum (M=d+1), normalize on DVE.
"""
import sys

for p in ("/opt/trn_rl_repo",):
    if p not in sys.path:
        sys.path.append(p)

import numpy as np

import concourse.bass as bass
import concourse.tile as tile
from concourse import bacc, mybir
from concourse.bass_utils import run_bass_kernel_spmd

FP32 = mybir.dt.float32
BF16 = mybir.dt.bfloat16
AF = mybir.ActivationFunctionType
ALU = mybir.AluOpType

T = 2048
C = 512
HS = 128          # heavy head size (= padded head size)
NT128 = T // 128  # 16
NT512 = T // 512  # 4
NCC = C // 128    # 4
SCALE = float(1.0 / np.sqrt(128.0))
SGRP = 3          # score tiles (512 wide) per exp group; 3 banks * 2 bufs + 2 = 8

_CACHE = {}


def _build():
    nc = bacc.Bacc("TRN2", target_bir_lowering=False, debug=False, num_devices=8)
    x_d = nc.dram_tensor("x", [T, C], FP32, kind="ExternalInput")
    w_d = nc.dram_tensor("w", [4, 1], FP32, kind="ExternalInput")
    bq_d = nc.dram_tensor("bq", [4, HS, C], FP32, kind="ExternalInput")
    bk_d = nc.dram_tensor("bk", [4, HS, C], FP32, kind="ExternalInput")
    bv_d = nc.dram_tensor("bv", [4, HS, C], FP32, kind="ExternalInput")
    out_d = nc.dram_tensor("out", [T, 4 * HS], FP32, kind="ExternalOutput")

    with tile.TileContext(nc) as tc:
        _emit(nc, tc, x_d, w_d, bq_d, bk_d, bv_d, out_d)
    nc.compile()
    return nc


def _emit(nc, tc, x_d, w_d, bq_d, bk_d, bv_d, out_d):
    from contextlib import ExitStack

    xbf_d = nc.dram_tensor("xbf_scratch", [T, C], BF16)

    ctx = ExitStack()
    with ctx:
        # ---- persistent SBUF pools ----
        const_p = ctx.enter_context(tc.tile_pool(name="const", bufs=1))
        wts_p = ctx.enter_context(tc.tile_pool(name="wts", bufs=1))
        stage_p = ctx.enter_context(tc.tile_pool(name="stage", bufs=3))
        xt_p = ctx.enter_context(tc.tile_pool(name="xt", bufs=1))
        qk_p = ctx.enter_context(tc.tile_pool(name="qk", bufs=1))
        v_p = ctx.enter_context(tc.tile_pool(name="v", bufs=1))
        pt_p = ctx.enter_context(tc.tile_pool(name="pt", bufs=1))
        o_p = ctx.enter_context(tc.tile_pool(name="o", bufs=3))
        r_p = ctx.enter_context(tc.tile_pool(name="r", bufs=3))
        # ---- PSUM pools: 3-bank score groups x2 + 1-bank small x2 = 8 banks
        sps = ctx.enter_context(tc.tile_pool(name="sps", bufs=2, space="PSUM"))
        ps = ctx.enter_context(tc.tile_pool(name="ps", bufs=2, space="PSUM"))

        # ================= constants =================
        ones_b = const_p.tile([128, C], BF16, tag="ones_b")
        nc.vector.memset(ones_b[:], 1.0)
        ident_b = const_p.tile([128, 128], BF16, tag="ident_b")
        nc.gpsimd.affine_select(
            ident_b[:], ones_b[:, 0:128], pattern=[[1, 128]],
            compare_op=ALU.is_equal, fill=0.0, base=0, channel_multiplier=-1)
        # causal masks for diagonal tiles: mask_r[s, t] = (t - s - 128*r >= 0)
        masks = []
        for r in range(4):
            m = const_p.tile([128, 512], BF16, tag=f"mask{r}")
            nc.gpsimd.affine_select(
                m[:], ones_b[:, 0:512], pattern=[[1, 512]],
                compare_op=ALU.is_ge, fill=0.0, base=-128 * r,
                channel_multiplier=-1)
            masks.append(m)

        # ================= eff patterns (bf16 rank-1 matmuls) ============
        # effA[d, e] = sum_i w_i * (d < hs_i) * (e < emb_i)         (heads 0-3)
        # effB[d, e] = same for i in {1,3} with (d%64 < hs_i)       (packed light)
        HSL = (64, 32, 128, 64)
        EMB = (256, 256, 512, 512)
        wsc = []
        for i in range(4):
            wi = const_p.tile([1, 1], FP32, name=f"wsc{i}", tag=f"wsc{i}")
            nc.sync.dma_start(wi[:], w_d.ap()[i:i + 1, :])
            wsc.append(wi)
        effA = const_p.tile([128, C], FP32, tag="effA")
        effB = const_p.tile([128, C], FP32, tag="effB")
        for eff, cfgs, ext in ((effA, (0, 1, 2, 3), False), (effB, (1, 3), True)):
            p = ps.tile([128, 512], FP32, tag="ps")
            for n, i in enumerate(cfgs):
                u = stage_p.tile([1, 128], BF16, name=f"u{i}{ext}", tag="u_row")
                nc.vector.memset(u[:], 0.0)
                if ext:  # packed light: both 64-halves get the (d%64 < hs) pattern
                    nc.vector.memset(u[0:1, 0:min(HSL[i], 64)], 1.0)
                    nc.vector.memset(u[0:1, 64:64 + min(HSL[i], 64)], 1.0)
                else:
                    nc.vector.memset(u[0:1, 0:HSL[i]], 1.0)
                uw = stage_p.tile([1, 128], BF16, name=f"uw{i}{ext}", tag="uw_row")
                nc.vector.tensor_scalar_mul(uw[:], u[:], wsc[i][:])
                vrow = stage_p.tile([1, C], BF16, name=f"v{i}{ext}", tag="v_row")
                nc.vector.memset(vrow[:], 0.0)
                nc.vector.memset(vrow[0:1, 0:EMB[i]], 1.0)
                nc.tensor.matmul(p[:], uw[:], vrow[:],
                                 start=(n == 0), stop=(n == len(cfgs) - 1))
            nc.vector.tensor_copy(eff[:], p[:])

        # ================= x -> bf16 -> x^T via DMA transpose ============
        for tt in range(NT128):
            xs = stage_p.tile([128, C], FP32, tag="xs")
            nc.sync.dma_start(xs[:], x_d.ap()[tt * 128:(tt + 1) * 128, :])
            xb = stage_p.tile([128, C], BF16, tag="xb")
            nc.vector.tensor_copy(xb[:], xs[:])
            nc.sync.dma_start(xbf_d.ap()[tt * 128:(tt + 1) * 128, :], xb[:])
        xt = [xt_p.tile([128, T], BF16, name=f"xt{cc}", tag=f"xt{cc}")
              for cc in range(NCC)]
        for cc in range(NCC):
            nc.sync.dma_start_transpose(
                xt[cc][:], xbf_d.ap()[:, cc * 128:(cc + 1) * 128])

        # ================= effective weights, transposed =================
        # wt[j][cc]: [128c, 128d] bf16, j in 0..8 (q h0,h1,l | k ... | v ...)
        wt = [[wts_p.tile([128, 128], BF16, name=f"wt{j}_{cc}", tag=f"wt{j}_{cc}")
               for cc in range(NCC)] for j in range(9)]
        for pi, bd in enumerate((bq_d, bk_d, bv_d)):
            for hj in range(3):
                j = pi * 3 + hj
                w_bf = stage_p.tile([128, C], BF16, tag="w_bf")
                base = stage_p.tile([128, C], FP32, tag="base")
                if hj < 2:
                    nc.sync.dma_start(base[:], bd.ap()[hj])
                    nc.vector.tensor_mul(w_bf[:], base[:], effA[:])
                else:
                    nc.sync.dma_start(base[0:64, :], bd.ap()[2][0:64, :])
                    nc.sync.dma_start(base[64:128, :], bd.ap()[3][0:64, :])
                    nc.vector.tensor_mul(w_bf[:], base[:], effB[:])
                for cc in range(NCC):
                    pt = ps.tile([128, 512], BF16, tag="ps")
                    nc.tensor.transpose(
                        pt[:, 0:128], w_bf[:, cc * 128:(cc + 1) * 128], ident_b[:])
                    nc.vector.tensor_copy(wt[j][cc][:], pt[:, 0:128])

        # ================= projections =================
        # q^T, k^T: [128d, T] bf16 per tensor (heavy0, heavy1, packed light)
        qt = [qk_p.tile([128, T], BF16, name=f"qt{h}", tag=f"qt{h}") for h in range(3)]
        kt = [qk_p.tile([128, T], BF16, name=f"kt{h}", tag=f"kt{h}") for h in range(3)]
        for dst, j0 in ((qt, 0), (kt, 3)):
            for hj in range(3):
                for tj in range(NT512):
                    p = ps.tile([128, 512], FP32, tag="ps")
                    for cc in range(NCC):
                        nc.tensor.matmul(
                            p[:], wt[j0 + hj][cc][:],
                            xt[cc][:, tj * 512:(tj + 1) * 512],
                            start=(cc == 0), stop=(cc == NCC - 1))
                    nc.scalar.copy(dst[hj][:, tj * 512:(tj + 1) * 512], p[:])
        # v tiles: heavy [128, 129] (ones col at 128); light packed [128, 130]
        # (head l0 cols 0:64 + ones at 64, head l1 cols 65:129 + ones at 129)
        vtiles = [[v_p.tile([128, 132], BF16, name=f"v{h}_{i}", tag=f"v{h}_{i}")
                   for i in range(NT128)] for h in range(3)]
        for hj in range(3):
            for i in range(NT128):
                p = ps.tile([128, 512], FP32, tag="ps")
                for cc in range(NCC):
                    nc.tensor.matmul(
                        p[:, 0:128], xt[cc][:, i * 128:(i + 1) * 128],
                        wt[6 + hj][cc][:],
                        start=(cc == 0), stop=(cc == NCC - 1))
                vt_i = vtiles[hj][i]
                if hj < 2:
                    nc.scalar.copy(vt_i[:, 0:128], p[:, 0:128])
                    nc.vector.memset(vt_i[:, 128:129], 1.0)
                else:
                    nc.scalar.copy(vt_i[:, 0:64], p[:, 0:64])
                    nc.scalar.copy(vt_i[:, 65:129], p[:, 64:128])
                    nc.vector.memset(vt_i[:, 64:65], 1.0)
                    nc.vector.memset(vt_i[:, 129:130], 1.0)

        # ================= attention =================
        # groups: (qk idx, [(sub, d_lo, d_hi, v_lo, v_hi, ocol), ...]); the two
        # light heads run together on disjoint PE row groups.
        ATT = [
            (0, [("A", 0, 128, 0, 129, 0)]),
            (1, [("A", 0, 128, 0, 129, 128)]),
            (2, [("A", 0, 64, 0, 65, 256), ("B", 64, 128, 65, 130, 384)]),
        ]
        for qkj, subs in ATT:
            vj = qkj
            for tj in range(NT512):
                S = 4 * tj + 4  # s-chunks (128 wide) needed for this t-chunk
                ptiles = {}
                for (sub, _, _, _, _, _) in subs:
                    ptiles[sub] = pt_p.tile(
                        [128, S * 512], BF16,
                        name=f"pt{sub}_{tj}", tag=f"pt{sub}_{tj}")
                g = 0
                while g < S:
                    gw = min(SGRP, S - g)
                    sp = {}
                    for (sub, d_lo, d_hi, _, _, _) in subs:
                        sp[sub] = sps.tile([128, SGRP * 512], FP32,
                                           name=f"sp{sub}", tag="sps")
                    for k in range(gw):
                        i = g + k
                        for (sub, d_lo, d_hi, _, _, _) in subs:
                            nc.tensor.matmul(
                                sp[sub][:, k * 512:(k + 1) * 512],
                                kt[qkj][d_lo:d_hi, i * 128:(i + 1) * 128],
                                qt[qkj][d_lo:d_hi, tj * 512:(tj + 1) * 512],
                                start=True, stop=True)
                    for (sub, d_lo, d_hi, _, _, _) in subs:
                        nc.scalar.activation(
                            ptiles[sub][:, g * 512:(g + gw) * 512],
                            sp[sub][:, 0:gw * 512], AF.Exp, scale=SCALE)
                    g += gw
                # causal masks on the 4 diagonal tiles
                for r in range(4):
                    i = 4 * tj + r
                    for (sub, _, _, _, _, _) in subs:
                        nc.vector.tensor_mul(
                            ptiles[sub][:, i * 512:(i + 1) * 512],
                            ptiles[sub][:, i * 512:(i + 1) * 512], masks[r][:])
                # AV + fused row-sum, normalize, store
                for m in range(4):
                    ti = 4 * tj + m  # global t128 index
                    for (sub, _, _, v_lo, v_hi, ocol) in subs:
                        w = v_hi - v_lo
                        op = ps.tile([128, 512], FP32, name=f"op{sub}", tag="ps")
                        for i in range(ti + 1):
                            nc.tensor.matmul(
                                op[:, 0:w],
                                ptiles[sub][:, i * 512 + m * 128:
                                            i * 512 + (m + 1) * 128],
                                vtiles[vj][i][:, v_lo:v_hi],
                                start=(i == 0), stop=(i == ti))
                        rec = r_p.tile([128, 1], FP32, name=f"rec{sub}",
                                       tag=f"rec{sub}")
                        nc.vector.reciprocal(rec[:], op[:, w - 1:w])
                        ob = o_p.tile([128, 128], FP32, name=f"ob{sub}",
                                      tag=f"ob{sub}")
                        nc.vector.tensor_scalar_mul(
                            ob[:, 0:w - 1], op[:, 0:w - 1], rec[:])
                        nc.sync.dma_start(
                            out_d.ap()[ti * 128:(ti + 1) * 128,
                                       ocol:ocol + (w - 1)],
                            ob[:, 0:w - 1])


def _shard_inputs(x, weights, base_K, base_Q, base_V):
    in_maps = []
    for c in range(8):
        b = c // 2
        hsel = [0, 1, 4, 5] if c % 2 == 0 else [2, 3, 6, 7]
        in_maps.append({
            "x": np.ascontiguousarray(x[b]),
            "w": np.ascontiguousarray(weights.reshape(4, 1)),
            "bq": np.ascontiguousarray(base_Q[hsel]),
            "bk": np.ascontiguousarray(base_K[hsel]),
            "bv": np.ascontiguousarray(base_V[hsel]),
        })
    return in_maps


def _gather(results):
    out = np.zeros((4, T, 8 * HS), np.float32)
    for c in range(8):
        o = results[c]["out"]
        hsel = [0, 1, 4, 5] if c % 2 == 0 else [2, 3, 6, 7]
        for j, h in enumerate(hsel):
            out[c // 2][:, h * HS:(h + 1) * HS] = o[:, j * HS:(j + 1) * HS]
    return out


def get_nc():
    if "nc" not in _CACHE:
        _CACHE["nc"] = _build()
    return _CACHE["nc"]


def kernel(x, weights, base_K, base_Q, base_V):
    x = np.asarray(x, np.float32)
    weights = np.asarray(weights, np.float32)
    base_K = np.asarray(base_K, np.float32)
    base_Q = np.asarray(base_Q, np.float32)
    base_V = np.asarray(base_V, np.float32)
    nc = get_nc()
    in_maps = _shard_inputs(x, weights, base_K, base_Q, base_V)
    res = run_bass_kernel_spmd(nc, in_maps, core_ids=list(range(8)))
    return _gather(res.results)
